# revision 1
# baseline (speedup 1.0000x reference)
"""Time-varying 33-tap FIR low-pass filter on 8 Trainium2 NeuronCores.

y[b,t] = sum_u filt[t,u] * x[b, t+u-16],  filt = host-computed windowed-sinc
bank (n,33) derived from scalars alpha/beta (tiny; O(n*33) host FLOPs).

Sharding: time dim split across the 8 cores (4096 t-columns each, all 64
batch rows).  Per core the banded matmul y = x @ W (contraction over input
time s) is tiled into 22 TensorE matmuls.  Each matmul packs TWO 128-sample
x-chunks, offset by 96 samples, side by side in the stationary operand
(K=128, M=128 = 2 halves x 64 batch).  The 96-offset makes every output
column's 33-tap band land entirely inside one half, so each PSUM column is
valid in exactly one 64-row half and the chunk serves 192 output columns
with no accumulation pass:

  lhsT[k, 64*h + b] = x[b, S + 96*h + k]           (S = core_t0 - 16 + 192*j)
  rhs [k, n]        = filt[S+16+n, u] at k = (n % 96) + u   (zeros elsewhere)
  psum[64*h(n) + b, n] = y[b, S+16+n],   h(n) = n // 96

Extraction: DVE copies PSUM->SBUF, then two DMAs pick the valid half-rows
(DMA cannot read PSUM on trn2).
"""

import sys
from contextlib import ExitStack

import numpy as np

if "/opt/trn_rl_repo" not in sys.path:
    sys.path.insert(0, "/opt/trn_rl_repo")

from concourse import bass, mybir
from concourse.bass_utils import run_bass_kernel_spmd

N = 32768          # time length
B = 64             # batch
NCORES = 8
TCORE = N // NCORES            # 4096 output columns per core
CT = 192                       # output columns served per chunk
NJ = (TCORE + CT - 1) // CT    # 22 chunks per core (last one partial: 64 cols)
KP = 128                       # contraction rows per matmul
TAPS = 33
HALF = 16

_prog_cache = None


def _filters_np(alpha, beta):
    """Numpy port of reference._filters (returns the flipped bank)."""
    t = np.arange(N, dtype=np.float64)
    cutoff = (np.pi / 4.0 + float(alpha) * np.sin(float(beta) * t / 8000.0)) / (
        2.0 * np.pi
    )
    k = np.arange(TAPS, dtype=np.float64)
    window = 0.5 - 0.5 * np.cos(2.0 * np.pi * k / (TAPS - 1.0))
    tvec = np.arange(-HALF, HALF + 1, dtype=np.float64)
    arg = 2.0 * np.pi * cutoff[:, None] * tvec[None, :]
    safe = np.where(arg == 0.0, 1.0, arg)
    sinc = np.where(arg == 0.0, 1.0, np.sin(safe) / safe)
    f = 2.0 * cutoff[:, None] * window[None, :] * sinc
    f = f / f.sum(axis=-1, keepdims=True)
    return np.ascontiguousarray(f[:, ::-1]).astype(np.float32)


def _prep_inputs(x, alpha, beta):
    """Build per-core stationary (xw) and banded-filter (wt) tiles."""
    filt = _filters_np(alpha, beta)  # (N, 33)

    pad = 16 + N + 512
    xp = np.zeros((B, pad), dtype=np.float32)
    xp[:, 16 : 16 + N] = x
    fp = np.zeros((N + 512, TAPS), dtype=np.float32)
    fp[:N] = filt

    c = np.arange(NCORES)[:, None, None, None]
    j = np.arange(NJ)[None, :, None, None]
    h = np.arange(2)[None, None, :, None]
    k = np.arange(KP)[None, None, None, :]
    # global s = TCORE*c - 16 + CT*j + 96*h + k ; +16 shifts into xp coords
    sidx = TCORE * c + CT * j + 96 * h + k
    xw = xp[:, sidx]  # (B, NCORES, NJ, 2, KP)
    xw = np.ascontiguousarray(
        np.transpose(xw, (1, 2, 4, 3, 0)).reshape(NCORES, NJ, KP, 128)
    )

    u = np.arange(TAPS)[:, None]  # (33, 1)
    nn = np.arange(CT)[None, :]  # (1, 192)
    rows = (nn % 96) + u  # (33, 192) target partition rows
    cols = np.broadcast_to(nn, (TAPS, CT))
    tg = (
        TCORE * np.arange(NCORES)[:, None, None]
        + CT * np.arange(NJ)[None, :, None]
        + np.arange(CT)[None, None, :]
    )  # (NCORES, NJ, 192) global output t per column
    vals = np.transpose(fp[tg], (0, 1, 3, 2))  # (NCORES, NJ, 33, 192)
    wt = np.zeros((NCORES, NJ, KP, CT), dtype=np.float32)
    wt[:, :, rows, cols] = vals

    # one combined [stationary | moving] tile per chunk -> one DMA, and the
    # self-loading fp32 matmul carries a single RAW wait (walrus limits the
    # sync-wait slots on InstMatmult)
    xwt = np.concatenate([xw, wt], axis=3)  # (NCORES, NJ, KP, 128 + CT)
    return np.ascontiguousarray(xwt)


OUT_GROUPS = (6, 12, 18, NJ)


def _build_program():
    """Raw Bass (no Tile): walrus permits a single sync-wait slot per Matmult
    and per DMA descriptor, so waits are emitted as standalone EventSemaphore
    instructions on each engine's queue instead."""
    nc = bass.Bass(trn_type="TRN2", debug=False)
    f32 = mybir.dt.float32
    W = 128 + CT  # 320 columns per combined [stationary | moving] chunk
    xwt_d = nc.dram_tensor("xwt", [NJ, KP, W], f32, kind="ExternalInput").ap()
    # raw staging dump (both PSUM halves); host picks the valid half per column
    y_d = nc.dram_tensor("yraw", [128, NJ * CT], f32, kind="ExternalOutput").ap()

    with ExitStack() as ctx:
        xts = ctx.enter_context(nc.sbuf_tensor("xts", [128, NJ * W], f32))
        st = ctx.enter_context(nc.sbuf_tensor("st", [128, NJ * CT], f32))
        pss = [
            ctx.enter_context(nc.psum_tensor(f"ps{i}", [128, 512], f32))
            for i in range(8)
        ]
        # qSPDynamicHW round-robins whole DMAs over 4 HW queues (+16 each);
        # completions reorder ACROSS queues but are FIFO within one, so pair
        # one semaphore per queue (sem = j%4) to make waits race-free
        NQ = 4
        sem_in = [ctx.enter_context(nc.semaphore(f"s_in{i}")) for i in range(NQ)]
        sem_pe = ctx.enter_context(nc.semaphore("s_pe"))
        sem_dve = ctx.enter_context(nc.semaphore("s_dve"))
        sem_out = [ctx.enter_context(nc.semaphore(f"s_out{i}")) for i in range(2)]
        block_cm = nc.Block()
        block = block_cm.__enter__()

        @block.sync
        def _(sync):
            for j in range(NJ):
                ins = sync.dma_start(out=xts[:, W * j : W * (j + 1)], in_=xwt_d[j])
                if j >= NQ:
                    # sem-reuse guard (free at runtime: same-queue FIFO)
                    ins.wait_op(sem_in[j % NQ], 16 * (j // NQ), "sem-ge")
                ins.then_inc(sem_in[j % NQ], 16)

        @block.tensor
        def _(tensor):
            for j in range(NJ):
                tensor.wait_ge(sem_in[j % NQ], 16 * (j // NQ + 1))
                if j >= 8:
                    # PSUM bank j%8 free once the copy of chunk j-8 retired
                    tensor.wait_ge(sem_dve, j - 7)
                tensor.matmul(
                    pss[j % 8].ap()[:, 0:CT],
                    xts[:, W * j : W * j + 128],
                    xts[:, W * j + 128 : W * (j + 1)],
                    start=True,
                    stop=True,
                ).then_inc(sem_pe, 1)

        @block.vector
        def _(vector):
            for j in range(NJ):
                vector.wait_ge(sem_pe, j + 1)
                vector.tensor_copy(
                    st[:, CT * j : CT * (j + 1)], pss[j % 8].ap()[:, 0:CT]
                ).then_inc(sem_dve, 1)

        @block.gpsimd
        def _(gpsimd):
            prev = 0
            for gi, gend in enumerate(OUT_GROUPS):
                gpsimd.wait_ge(sem_dve, gend)
                ins = gpsimd.dma_start(
                    out=y_d[:, CT * prev : CT * gend],
                    in_=st[:, CT * prev : CT * gend],
                )
                if gi >= 2:
                    ins.wait_op(sem_out[gi % 2], 16, "sem-ge")
                ins.then_inc(sem_out[gi % 2], 16)
                prev = gend
            # SWDGE (qPoolDynamic) DMAs don't fan out: +16 per DMA
            for i in range(2):
                gpsimd.wait_ge(sem_out[i], 16 * (len(OUT_GROUPS) // 2))

        block_cm.__exit__(None, None, None)  # all-engine exit barrier

        # zero the semaphores after the barrier so re-executing the same NEFF
        # starts from a clean state
        with nc.Block() as block2:

            @block2.gpsimd
            def _(gpsimd):
                for sem in (*sem_in, sem_pe, sem_dve, *sem_out):
                    gpsimd.sem_clear(sem)

    return nc


def run_sharded(inputs, trace=False):
    global _prog_cache
    x = np.ascontiguousarray(np.asarray(inputs["input"], dtype=np.float32))
    xwt = _prep_inputs(x, inputs["alpha"], inputs["beta"])
    if _prog_cache is None:
        _prog_cache = _build_program()
    nc = _prog_cache
    in_maps = [{"xwt": xwt[cc]} for cc in range(NCORES)]
    res = run_bass_kernel_spmd(nc, in_maps, list(range(NCORES)), trace=trace)
    shards = []
    for cc in range(NCORES):
        raw = res.results[cc]["yraw"].reshape(128, NJ, CT)
        sel = np.empty((B, NJ, CT), dtype=np.float32)
        sel[:, :, 0:96] = raw[0:64, :, 0:96]
        sel[:, :, 96:CT] = raw[64:128, :, 96:CT]
        shards.append(sel.reshape(B, NJ * CT)[:, :TCORE])
    y = np.concatenate(shards, axis=1)
    return y, res


def kernel(input, alpha, beta):
    y, _ = run_sharded({"input": input, "alpha": alpha, "beta": beta})
    return y



# revision 2
# speedup vs baseline: 1.5426x; 1.5426x over previous
"""Time-varying 33-tap FIR low-pass filter on 8 Trainium2 NeuronCores.

y[b,t] = sum_u filt[t,u] * x[b, t+u-16],  filt = host-computed windowed-sinc
bank (n,33) derived from scalars alpha/beta (tiny; O(n*33) host FLOPs).

Sharding: time dim split across the 8 cores (4096 t-columns each, all 64
batch rows).  Per core the banded matmul y = x @ W (contraction over input
time s) is tiled into 22 TensorE matmuls.  Each matmul packs TWO 128-sample
x-chunks, offset by 96 samples, side by side in the stationary operand
(K=128, M=128 = 2 halves x 64 batch).  The 96-offset makes every output
column's 33-tap band land entirely inside one half, so each PSUM column is
valid in exactly one 64-row half and the chunk serves 192 output columns
with no accumulation pass:

  lhsT[k, 64*h + b] = x[b, S + 96*h + k]           (S = core_t0 - 16 + 192*j)
  rhs [k, n]        = filt[S+16+n, u] at k = (n % 96) + u   (zeros elsewhere)
  psum[64*h(n) + b, n] = y[b, S+16+n],   h(n) = n // 96

v2 (perf): all matmul operands in bf16 (fp32 matmul runs 4 cycles/row on
TRN2, bf16 runs 1; input DMA bytes halve).  Input is laid out partition-
major ([128, NJ*W]) so the 6 grouped input DMAs move long (>=1280B) lines
at the full 360GB/s descriptor rate.  The valid PSUM half-rows are
extracted on-device -- DVE copies half0 (cols 0:96 from partitions 0:64),
the Activation engine copies half1 (cols 96:192 from partitions 64:128) --
converting to bf16 into a compact [64, NJ*192] staging tile, so the output
DMA moves 1/4 of the baseline bytes.  Host upcasts to fp32.
"""

import sys
from contextlib import ExitStack

import numpy as np
import ml_dtypes

if "/opt/trn_rl_repo" not in sys.path:
    sys.path.insert(0, "/opt/trn_rl_repo")

from concourse import bass, mybir
from concourse.bass_utils import run_bass_kernel_spmd

N = 32768          # time length
B = 64             # batch
NCORES = 8
TCORE = N // NCORES            # 4096 output columns per core
CT = 192                       # output columns served per chunk
NJ = (TCORE + CT - 1) // CT    # 22 chunks per core (last one partial: 64 cols)
KP = 128                       # contraction rows per matmul
TAPS = 33
HALF = 16
W = 128 + CT                   # 320 columns per [stationary | moving] chunk

# input DMA groups (chunk boundaries): first group small so compute starts
# early, then 4-chunk groups whose 2560B lines run at full descriptor rate
IN_GROUPS = (2, 6, 10, 14, 18, NJ)
OUT_GROUPS = (6, 12, 18, NJ)

_prog_cache = None


def _filters_np(alpha, beta):
    """Numpy port of reference._filters (returns the flipped bank)."""
    t = np.arange(N, dtype=np.float64)
    cutoff = (np.pi / 4.0 + float(alpha) * np.sin(float(beta) * t / 8000.0)) / (
        2.0 * np.pi
    )
    k = np.arange(TAPS, dtype=np.float64)
    window = 0.5 - 0.5 * np.cos(2.0 * np.pi * k / (TAPS - 1.0))
    tvec = np.arange(-HALF, HALF + 1, dtype=np.float64)
    arg = 2.0 * np.pi * cutoff[:, None] * tvec[None, :]
    safe = np.where(arg == 0.0, 1.0, arg)
    sinc = np.where(arg == 0.0, 1.0, np.sin(safe) / safe)
    f = 2.0 * cutoff[:, None] * window[None, :] * sinc
    f = f / f.sum(axis=-1, keepdims=True)
    return np.ascontiguousarray(f[:, ::-1]).astype(np.float32)


def _prep_inputs(x, alpha, beta):
    """Build per-core [KP, NJ*W] bf16 [stationary | banded-filter] tiles."""
    filt = _filters_np(alpha, beta)  # (N, 33)

    pad = 16 + N + 512
    xp = np.zeros((B, pad), dtype=np.float32)
    xp[:, 16 : 16 + N] = x
    xp = xp.astype(ml_dtypes.bfloat16)
    fp = np.zeros((N + 512, TAPS), dtype=ml_dtypes.bfloat16)
    fp[:N] = filt.astype(ml_dtypes.bfloat16)

    c = np.arange(NCORES)[:, None, None, None]
    j = np.arange(NJ)[None, :, None, None]
    h = np.arange(2)[None, None, :, None]
    k = np.arange(KP)[None, None, None, :]
    # global s = TCORE*c - 16 + CT*j + 96*h + k ; +16 shifts into xp coords
    sidx = TCORE * c + CT * j + 96 * h + k
    xw = xp[:, sidx]  # (B, NCORES, NJ, 2, KP)
    xw = np.ascontiguousarray(
        np.transpose(xw, (1, 2, 4, 3, 0)).reshape(NCORES, NJ, KP, 128)
    )

    u = np.arange(TAPS)[:, None]  # (33, 1)
    nn = np.arange(CT)[None, :]  # (1, 192)
    rows = (nn % 96) + u  # (33, 192) target partition rows
    cols = np.broadcast_to(nn, (TAPS, CT))
    tg = (
        TCORE * np.arange(NCORES)[:, None, None]
        + CT * np.arange(NJ)[None, :, None]
        + np.arange(CT)[None, None, :]
    )  # (NCORES, NJ, 192) global output t per column
    vals = np.transpose(fp[tg], (0, 1, 3, 2))  # (NCORES, NJ, 33, 192)
    wt = np.zeros((NCORES, NJ, KP, CT), dtype=ml_dtypes.bfloat16)
    wt[:, :, rows, cols] = vals

    # one combined [stationary | moving] tile per chunk, then partition-major
    # ([KP, NJ, W]) so grouped input DMAs move long contiguous lines
    xwt = np.concatenate([xw, wt], axis=3)  # (NCORES, NJ, KP, W)
    xwt = np.transpose(xwt, (0, 2, 1, 3)).reshape(NCORES, KP, NJ * W)
    return np.ascontiguousarray(xwt)


def _build_program():
    """Raw Bass (no Tile): walrus permits a single sync-wait slot per Matmult
    and per DMA descriptor, so waits are emitted as standalone EventSemaphore
    instructions on each engine's queue instead."""
    nc = bass.Bass(trn_type="TRN2", debug=False)
    f32 = mybir.dt.float32
    bf16 = mybir.dt.bfloat16
    xwt_d = nc.dram_tensor("xwt", [KP, NJ * W], bf16, kind="ExternalInput").ap()
    y_d = nc.dram_tensor("y", [B, NJ * CT], bf16, kind="ExternalOutput").ap()

    def grp(j):
        """Input-group index of chunk j."""
        for g, gend in enumerate(IN_GROUPS):
            if j < gend:
                return g

    with ExitStack() as ctx:
        xts = ctx.enter_context(nc.sbuf_tensor("xts", [KP, NJ * W], bf16))
        st = ctx.enter_context(nc.sbuf_tensor("st", [B, NJ * CT], bf16))
        pss = [
            ctx.enter_context(nc.psum_tensor(f"ps{i}", [128, 512], f32))
            for i in range(8)
        ]
        # qSPDynamicHW round-robins whole DMAs over 4 HW queues (+16 each);
        # completions reorder ACROSS queues but are FIFO within one, so pair
        # one semaphore per queue (sem = g%4) to make waits race-free
        NQ = 4
        sem_in = [ctx.enter_context(nc.semaphore(f"s_in{i}")) for i in range(NQ)]
        sem_pe = ctx.enter_context(nc.semaphore("s_pe"))
        sem_dve = ctx.enter_context(nc.semaphore("s_dve"))
        sem_act = ctx.enter_context(nc.semaphore("s_act"))
        sem_out = [
            ctx.enter_context(nc.semaphore(f"s_out{i}"))
            for i in range(len(OUT_GROUPS))
        ]
        block_cm = nc.Block()
        block = block_cm.__enter__()

        @block.sync
        def _(sync):
            prev = 0
            for g, gend in enumerate(IN_GROUPS):
                ins = sync.dma_start(
                    out=xts[:, W * prev : W * gend], in_=xwt_d[:, W * prev : W * gend]
                )
                if g >= NQ:
                    # sem-reuse guard (free at runtime: same-queue FIFO)
                    ins.wait_op(sem_in[g % NQ], 16 * (g // NQ), "sem-ge")
                ins.then_inc(sem_in[g % NQ], 16)
                prev = gend

        @block.tensor
        def _(tensor):
            for j in range(NJ):
                g = grp(j)
                tensor.wait_ge(sem_in[g % NQ], 16 * (g // NQ + 1))
                if j >= 8:
                    # PSUM bank j%8 free once BOTH half-copies of j-8 retired
                    tensor.wait_ge(sem_dve, j - 7)
                    tensor.wait_ge(sem_act, j - 7)
                tensor.matmul(
                    pss[j % 8].ap()[:, 0:CT],
                    xts[:, W * j : W * j + 128],
                    xts[:, W * j + 128 : W * (j + 1)],
                    start=True,
                    stop=True,
                ).then_inc(sem_pe, 1)

        @block.vector
        def _(vector):
            # half0: outputs 0:96 of each chunk live in PSUM partitions 0:64
            for j in range(NJ):
                vector.wait_ge(sem_pe, j + 1)
                vector.tensor_copy(
                    st[0:B, CT * j : CT * j + 96], pss[j % 8].ap()[0:B, 0:96]
                ).then_inc(sem_dve, 1)

        @block.scalar
        def _(scalar):
            # half1: outputs 96:192 of each chunk live in PSUM partitions 64:128
            for j in range(NJ):
                scalar.wait_ge(sem_pe, j + 1)
                scalar.copy(
                    st[0:B, CT * j + 96 : CT * (j + 1)],
                    pss[j % 8].ap()[B : 2 * B, 96:CT],
                ).then_inc(sem_act, 1)

        @block.gpsimd
        def _(gpsimd):
            prev = 0
            for gi, gend in enumerate(OUT_GROUPS):
                gpsimd.wait_ge(sem_dve, gend)
                gpsimd.wait_ge(sem_act, gend)
                gpsimd.dma_start(
                    out=y_d[:, CT * prev : CT * gend],
                    in_=st[:, CT * prev : CT * gend],
                ).then_inc(sem_out[gi], 16)
                prev = gend
            # SWDGE (qPoolDynamic) DMAs don't fan out: +16 per DMA
            for i in range(len(OUT_GROUPS)):
                gpsimd.wait_ge(sem_out[i], 16)

        block_cm.__exit__(None, None, None)  # all-engine exit barrier

        # zero the semaphores after the barrier so re-executing the same NEFF
        # starts from a clean state
        with nc.Block() as block2:

            @block2.gpsimd
            def _(gpsimd):
                for sem in (*sem_in, sem_pe, sem_dve, sem_act, *sem_out):
                    gpsimd.sem_clear(sem)

    return nc


def run_sharded(inputs, trace=False):
    global _prog_cache
    x = np.ascontiguousarray(np.asarray(inputs["input"], dtype=np.float32))
    xwt = _prep_inputs(x, inputs["alpha"], inputs["beta"])
    if _prog_cache is None:
        _prog_cache = _build_program()
    nc = _prog_cache
    in_maps = [{"xwt": xwt[cc]} for cc in range(NCORES)]
    res = run_bass_kernel_spmd(nc, in_maps, list(range(NCORES)), trace=trace)
    shards = [
        res.results[cc]["y"][:, :TCORE].astype(np.float32) for cc in range(NCORES)
    ]
    y = np.concatenate(shards, axis=1)
    return y, res


def kernel(input, alpha, beta):
    y, _ = run_sharded({"input": input, "alpha": alpha, "beta": beta})
    return y


# revision 7
# speedup vs baseline: 1.6494x; 1.0692x over previous
"""Time-varying 33-tap FIR low-pass filter on 8 Trainium2 NeuronCores.

y[b,t] = sum_u filt[t,u] * x[b, t+u-16],  filt = host-computed windowed-sinc
bank (n,33) derived from scalars alpha/beta (tiny; O(n*33) host FLOPs).

Sharding: time dim split across the 8 cores (4096 t-columns each, all 64
batch rows).  Per core the banded matmul y = x @ W (contraction over input
time s) is tiled into 22 TensorE matmuls.  Each matmul packs TWO 128-sample
x-chunks, offset by 96 samples, side by side in the stationary operand
(K=128, M=128 = 2 halves x 64 batch).  The 96-offset makes every output
column's 33-tap band land entirely inside one half, so each PSUM column is
valid in exactly one 64-row half and the chunk serves 192 output columns
with no accumulation pass:

  lhsT[k, 64*h + b] = x[b, S + 96*h + k]           (S = core_t0 - 16 + 192*j)
  rhs [k, n]        = filt[S+16+n, u] at k = (n % 96) + u   (zeros elsewhere)
  psum[64*h(n) + b, n] = y[b, S+16+n],   h(n) = n // 96

v2 (perf): all matmul operands in bf16 (fp32 matmul runs 4 cycles/row on
TRN2, bf16 runs 1; input DMA bytes halve).  Input is laid out partition-
major ([128, NJ*W]) so the 6 grouped input DMAs move long (>=1280B) lines
at the full 360GB/s descriptor rate.  The valid PSUM half-rows are
extracted on-device -- DVE copies half0 (cols 0:96 from partitions 0:64),
the Activation engine copies half1 (cols 96:192 from partitions 64:128) --
converting to bf16 into a compact [64, NJ*192] staging tile, so the output
DMA moves 1/4 of the baseline bytes.  Host upcasts to fp32.
"""

import sys
from contextlib import ExitStack

import numpy as np
import ml_dtypes

if "/opt/trn_rl_repo" not in sys.path:
    sys.path.insert(0, "/opt/trn_rl_repo")

from concourse import bass, mybir
from concourse.bass_utils import run_bass_kernel_spmd

N = 32768          # time length
B = 64             # batch
NCORES = 8
TCORE = N // NCORES            # 4096 output columns per core
CT = 192                       # output columns served per chunk
NJ = (TCORE + CT - 1) // CT    # 22 chunks per core (last one partial: 64 cols)
KP = 128                       # contraction rows per matmul
TAPS = 33
HALF = 16
W = 128 + CT                   # 320 columns per [stationary | moving] chunk

# input DMA groups (chunk boundaries): first group small so compute starts
# early, then 4-chunk groups whose 2560B lines run at full descriptor rate
IN_GROUPS = (2, 6, 10, 14, 18, NJ)
# output DMA groups in PAIR units (extraction is pair-batched).  Biased late:
# the DMA device serializes transfers, so early output groups would delay the
# input stream that compute is waiting on.  Last group small for a short tail.
OUT_GROUPS_P = (5, 8, 10, 11)

_prog_cache = None


def _filters_np(alpha, beta):
    """Numpy port of reference._filters (returns the flipped bank)."""
    t = np.arange(N, dtype=np.float64)
    cutoff = (np.pi / 4.0 + float(alpha) * np.sin(float(beta) * t / 8000.0)) / (
        2.0 * np.pi
    )
    k = np.arange(TAPS, dtype=np.float64)
    window = 0.5 - 0.5 * np.cos(2.0 * np.pi * k / (TAPS - 1.0))
    tvec = np.arange(-HALF, HALF + 1, dtype=np.float64)
    arg = 2.0 * np.pi * cutoff[:, None] * tvec[None, :]
    safe = np.where(arg == 0.0, 1.0, arg)
    sinc = np.where(arg == 0.0, 1.0, np.sin(safe) / safe)
    f = 2.0 * cutoff[:, None] * window[None, :] * sinc
    f = f / f.sum(axis=-1, keepdims=True)
    return np.ascontiguousarray(f[:, ::-1]).astype(np.float32)


def _prep_inputs(x, alpha, beta):
    """Build per-core [KP, NJ*W] bf16 [stationary | banded-filter] tiles."""
    filt = _filters_np(alpha, beta)  # (N, 33)

    pad = 16 + N + 512
    xp = np.zeros((B, pad), dtype=np.float32)
    xp[:, 16 : 16 + N] = x
    xp = xp.astype(ml_dtypes.bfloat16)
    fp = np.zeros((N + 512, TAPS), dtype=ml_dtypes.bfloat16)
    fp[:N] = filt.astype(ml_dtypes.bfloat16)

    c = np.arange(NCORES)[:, None, None, None]
    j = np.arange(NJ)[None, :, None, None]
    h = np.arange(2)[None, None, :, None]
    k = np.arange(KP)[None, None, None, :]
    # global s = TCORE*c - 16 + CT*j + 96*h + k ; +16 shifts into xp coords
    sidx = TCORE * c + CT * j + 96 * h + k
    xw = xp[:, sidx]  # (B, NCORES, NJ, 2, KP)
    xw = np.ascontiguousarray(
        np.transpose(xw, (1, 2, 4, 3, 0)).reshape(NCORES, NJ, KP, 128)
    )

    u = np.arange(TAPS)[:, None]  # (33, 1)
    nn = np.arange(CT)[None, :]  # (1, 192)
    rows = (nn % 96) + u  # (33, 192) target partition rows
    cols = np.broadcast_to(nn, (TAPS, CT))
    tg = (
        TCORE * np.arange(NCORES)[:, None, None]
        + CT * np.arange(NJ)[None, :, None]
        + np.arange(CT)[None, None, :]
    )  # (NCORES, NJ, 192) global output t per column
    vals = np.transpose(fp[tg], (0, 1, 3, 2))  # (NCORES, NJ, 33, 192)
    wt = np.zeros((NCORES, NJ, KP, CT), dtype=ml_dtypes.bfloat16)
    wt[:, :, rows, cols] = vals

    # one combined [stationary | moving] tile per chunk, then partition-major
    # ([KP, NJ, W]) so grouped input DMAs move long contiguous lines
    xwt = np.concatenate([xw, wt], axis=3)  # (NCORES, NJ, KP, W)
    xwt = np.transpose(xwt, (0, 2, 1, 3)).reshape(NCORES, KP, NJ * W)
    return np.ascontiguousarray(xwt)


def _build_program():
    """Raw Bass (no Tile): walrus permits a single sync-wait slot per Matmult
    and per DMA descriptor, so waits are emitted as standalone EventSemaphore
    instructions on each engine's queue instead."""
    nc = bass.Bass(trn_type="TRN2", debug=False)
    f32 = mybir.dt.float32
    bf16 = mybir.dt.bfloat16
    xwt_d = nc.dram_tensor("xwt", [KP, NJ * W], bf16, kind="ExternalInput").ap()
    y_d = nc.dram_tensor("y", [B, NJ * CT], bf16, kind="ExternalOutput").ap()

    def grp(j):
        """Input-group index of chunk j."""
        for g, gend in enumerate(IN_GROUPS):
            if j < gend:
                return g

    with ExitStack() as ctx:
        xts = ctx.enter_context(nc.sbuf_tensor("xts", [KP, NJ * W], bf16))
        st = ctx.enter_context(nc.sbuf_tensor("st", [B, NJ, CT], bf16))
        # 4 pair-tensors of 2 PSUM banks each (slot = 512 fp32 = one bank);
        # extraction reads both slots of a pair in one 3D-AP op
        pps = [
            ctx.enter_context(nc.psum_tensor(f"pp{i}", [128, 2, 512], f32))
            for i in range(4)
        ]
        # qSPDynamicHW round-robins whole DMAs over 4 HW queues (+16 each);
        # completions reorder ACROSS queues but are FIFO within one, so pair
        # one semaphore per queue (sem = g%4) to make waits race-free
        NQ = 4
        sem_in = [ctx.enter_context(nc.semaphore(f"s_in{i}")) for i in range(NQ)]
        sem_pe = ctx.enter_context(nc.semaphore("s_pe"))
        sem_dve = ctx.enter_context(nc.semaphore("s_dve"))
        sem_act = ctx.enter_context(nc.semaphore("s_act"))
        sem_out = [
            ctx.enter_context(nc.semaphore(f"s_out{i}"))
            for i in range(len(OUT_GROUPS_P))
        ]
        block_cm = nc.Block()
        block = block_cm.__enter__()

        @block.sync
        def _(sync):
            prev = 0
            for g, gend in enumerate(IN_GROUPS):
                ins = sync.dma_start(
                    out=xts[:, W * prev : W * gend], in_=xwt_d[:, W * prev : W * gend]
                )
                if g >= NQ:
                    # sem-reuse guard (free at runtime: same-queue FIFO)
                    ins.wait_op(sem_in[g % NQ], 16 * (g // NQ), "sem-ge")
                ins.then_inc(sem_in[g % NQ], 16)
                prev = gend

        @block.tensor
        def _(tensor):
            for j in range(NJ):
                g = grp(j)
                if j == 0 or grp(j - 1) != g:
                    tensor.wait_ge(sem_in[g % NQ], 16 * (g // NQ + 1))
                if j >= 8 and j % 2 == 0:
                    # PSUM pair (j//2)%4 free once BOTH pair-copies of pair
                    # j//2-4 retired (odd j reuses the same pair: no new wait)
                    tensor.wait_ge(sem_dve, j // 2 - 3)
                    tensor.wait_ge(sem_act, j // 2 - 3)
                tensor.matmul(
                    pps[(j // 2) % 4].ap()[:, j % 2, 0:CT],
                    xts[:, W * j : W * j + 128],
                    xts[:, W * j + 128 : W * (j + 1)],
                    start=True,
                    stop=True,
                ).then_inc(sem_pe, 1)

        @block.vector
        def _(vector):
            # half0: outputs 0:96 of each chunk live in PSUM partitions 0:64;
            # one 3D-AP op extracts both chunks of a pair
            for p in range(NJ // 2):
                vector.wait_ge(sem_pe, 2 * p + 2)
                vector.tensor_copy(
                    st[0:B, 2 * p : 2 * p + 2, 0:96],
                    pps[p % 4].ap()[0:B, 0:2, 0:96],
                ).then_inc(sem_dve, 1)

        @block.scalar
        def _(scalar):
            # half1: outputs 96:192 of each chunk live in PSUM partitions 64:128
            for p in range(NJ // 2):
                scalar.wait_ge(sem_pe, 2 * p + 2)
                scalar.copy(
                    st[0:B, 2 * p : 2 * p + 2, 96:CT],
                    pps[p % 4].ap()[B : 2 * B, 0:2, 96:CT],
                ).then_inc(sem_act, 1)

        @block.gpsimd
        def _(gpsimd):
            prev = 0
            for gi, pend in enumerate(OUT_GROUPS_P):
                gpsimd.wait_ge(sem_dve, pend)
                gpsimd.wait_ge(sem_act, pend)
                gpsimd.dma_start(
                    out=y_d[:, 2 * CT * prev : 2 * CT * pend],
                    in_=st[:, 2 * prev : 2 * pend, :],
                ).then_inc(sem_out[gi], 16)
                prev = pend
            # SWDGE (qPoolDynamic) DMAs don't fan out: +16 per DMA
            for i in range(len(OUT_GROUPS_P)):
                gpsimd.wait_ge(sem_out[i], 16)

        block_cm.__exit__(None, None, None)  # all-engine exit barrier

        # zero the semaphores after the barrier so re-executing the same NEFF
        # starts from a clean state
        with nc.Block() as block2:

            @block2.gpsimd
            def _(gpsimd):
                for sem in (*sem_in, sem_pe, sem_dve, sem_act, *sem_out):
                    gpsimd.sem_clear(sem)

    return nc


def run_sharded(inputs, trace=False):
    global _prog_cache
    x = np.ascontiguousarray(np.asarray(inputs["input"], dtype=np.float32))
    xwt = _prep_inputs(x, inputs["alpha"], inputs["beta"])
    if _prog_cache is None:
        _prog_cache = _build_program()
    nc = _prog_cache
    in_maps = [{"xwt": xwt[cc]} for cc in range(NCORES)]
    res = run_bass_kernel_spmd(nc, in_maps, list(range(NCORES)), trace=trace)
    shards = [
        res.results[cc]["y"][:, :TCORE].astype(np.float32) for cc in range(NCORES)
    ]
    y = np.concatenate(shards, axis=1)
    return y, res


def kernel(input, alpha, beta):
    y, _ = run_sharded({"input": input, "alpha": alpha, "beta": beta})
    return y
